# revision 3
# baseline (speedup 1.0000x reference)
"""Trainium2 Bass kernel for Conv2Demod (StyleGAN2-style modulated conv).

Reference computation (per sample b):
    w[b,o,i,ky,kx] = weight[o,i,ky,kx] * (1 + s[b,i])
    d[b,o]         = rsqrt(sum_{i,ky,kx} w^2 + 1e-8)
    out[b]         = conv2d(img[b], w[b]*d[b,o], pad=1)

v2: 1-D Winograd F(2,3) along y cuts PE work 1.5x vs direct conv
(768 vs 1152 N=512 matmuls/sample). All matmul operands are bf16
(full PE rate, FWL weight loads); accumulation stays fp32 in PSUM.

Per-sample algebra:
  - modulation (1+s[i]) folded into the image (per-partition scale, DVE),
  - demodulation d[o] folded into PSUM eviction (per-partition scale, ACT),
  - d[o] from the tiny matvec  A_T[i,o] @ (1+s[i])^2  (A_T host-computed),
  - y-transform: V[a] = (B^T img)[a] with B^T rows {r0-r2, r1+r2, r2-r1,
    r1-r3} over row pairs at stride 2 (4 DVE tensor_tensor ops/i-tile),
  - x-direction stays a direct 3-tap conv via shifted rhs reads,
  - U[a,kx,i,o] = sum_ky G[a,ky] w[o,i,ky,kx] host-precomputed in bf16,
  - inverse transform: y0 = m0+m1+m2, y1 = m1-m2-m3 (4 DVE ops/chunk),
    so output rows interleave as (2*ty, 2*ty+1) -> contiguous DMA.

Sharding: data-parallel over batch -- 8 samples onto 8 NeuronCores.
"""

import contextlib

import numpy as np
import ml_dtypes

import concourse.bacc as bacc
import concourse.mybir as mybir
import concourse.tile as tile
from concourse.bass_utils import run_bass_kernel_spmd

P = 128          # partitions
CIN = 512
COUT = 512
H = W = 64
KS = 3
NA = 4           # y-transform positions
NK = NA * KS     # 12 U planes
NI = CIN // P    # 4 i-tiles
NO = COUT // P   # 4 o-tiles
TY = 32          # output row-tiles (2 rows each)
TCH = 4          # tile-row chunks
TROWS = TY // TCH  # 8 tile-rows per chunk
NPIX = TROWS * W   # 512 = matmul N
HP = H + 2
WP = W + 2
EPS = 1e-8
N_CORES = 8

F32 = mybir.dt.float32
BF16 = mybir.dt.bfloat16
AF = mybir.ActivationFunctionType
OP = mybir.AluOpType
_nullcm = contextlib.nullcontext


def build_nc(chain=False, loop_n=None):
    """Per-core program: one sample's modulated conv via 1-D Winograd."""
    nc = bacc.Bacc("TRN2", target_bir_lowering=False, debug=False)

    # host-padded bf16 image: [i_tile, partition, 66, 66] with zero border
    img = nc.dram_tensor("img", [NI, P, HP, WP], BF16, kind="ExternalInput").ap()
    s_in = nc.dram_tensor("s", [CIN], F32, kind="ExternalInput").ap()
    ut = nc.dram_tensor("ut", [NK, CIN, COUT], BF16, kind="ExternalInput").ap()
    at = nc.dram_tensor("at", [CIN, COUT], BF16, kind="ExternalInput").ap()
    out = nc.dram_tensor("out", [COUT, H, W], F32, kind="ExternalOutput").ap()
    s_out = None
    if chain:
        s_out = nc.dram_tensor("s_out", [CIN], F32, kind="ExternalOutput").ap()

    with tile.TileContext(nc) as tc:
        with (
            tc.tile_pool(name="const", bufs=1) as cpool,
            tc.tile_pool(name="persist", bufs=1) as ppool,
            tc.tile_pool(name="imgst", bufs=2) as imgst,
            tc.tile_pool(name="mpool", bufs=3) as mpool,
            tc.tile_pool(name="tpool", bufs=4) as tpool,
            tc.tile_pool(name="ypool", bufs=3) as ypool,
            tc.tile_pool(name="psum", bufs=7, space="PSUM") as psum_pool,
            tc.tile_pool(name="psum_d", bufs=1, space="PSUM") as psum_d,
        ):
            with (tc.For_i(0, loop_n, 1) if loop_n else _nullcm()):
                # ---- s-derived scalars -------------------------------------
                sraw = cpool.tile([P, NI, 2], F32, tag="sraw")
                for c in range(2):
                    nc.sync.dma_start(
                        sraw[:, :, c], s_in.rearrange("(t p) -> p t", p=P)
                    )
                if chain:
                    nc.sync.dma_start(s_out[:], s_in[:])
                smod = cpool.tile([P, NI, 2], F32, tag="smod")  # 1 + s
                nc.scalar.activation(smod[:], sraw[:], AF.Copy, bias=1.0)
                tsq = cpool.tile([P, NI, 2], BF16, tag="tsq")   # (1 + s)^2
                nc.scalar.square(tsq[:], smod[:])

                # ---- demod d[o] = 1/sqrt(A_T.T @ tsq + eps) ----------------
                at_sb = ppool.tile([P, NI, COUT], BF16, tag="at_sb")
                nc.sync.dma_start(at_sb[:], at.rearrange("(t p) o -> p t o", p=P))
                dsb = cpool.tile([P, NO], F32, tag="dsb")
                dtmp = cpool.tile([P, NO], F32, tag="dtmp")
                epst = cpool.tile([P, 1], F32, tag="epst")
                nc.vector.memset(epst[:], EPS)
                for ot in range(NO):
                    o0 = ot * P
                    psd = psum_d.tile([P, 2], F32)
                    for it in range(NI):
                        nc.tensor.matmul(
                            psd[:],
                            at_sb[:, it, o0 : o0 + P],
                            tsq[:, it, :],
                            start=(it == 0),
                            stop=(it == NI - 1),
                        )
                    nc.scalar.activation(
                        dtmp[:, ot : ot + 1], psd[:, 0:1], AF.Sqrt, bias=epst[:]
                    )
                nc.vector.reciprocal(dsb[:], dtmp[:])

                # ---- modulated y-transformed image V[a] --------------------
                vsb = []
                for it in range(NI):
                    t = ppool.tile([P, NA, TY, WP], BF16, tag=f"vsb{it}")
                    vsb.append(t)
                for it in range(NI):
                    ir = imgst.tile([P, HP, WP], BF16)
                    nc.sync.dma_start(ir[:], img[it])
                    # modulation: img *= (1 + s[i]) in-place (DVE 4x bf16)
                    nc.vector.tensor_scalar_mul(ir[:], ir[:], smod[:, it, 0:1])
                    v = vsb[it]
                    r0 = ir[:, 0 : 2 * TY : 2, :]
                    r1 = ir[:, 1 : 2 * TY + 1 : 2, :]
                    r2 = ir[:, 2 : 2 * TY + 2 : 2, :]
                    r3 = ir[:, 3 : 2 * TY + 2 : 2, :]
                    nc.vector.tensor_tensor(v[:, 0], r0, r2, OP.subtract)
                    nc.vector.tensor_tensor(v[:, 1], r1, r2, OP.add)
                    nc.vector.tensor_tensor(v[:, 2], r2, r1, OP.subtract)
                    nc.vector.tensor_tensor(v[:, 3], r1, r3, OP.subtract)

                # ---- transformed weights U ---------------------------------
                usb = []
                for it in range(NI):
                    t = ppool.tile([P, NK, COUT], BF16, tag=f"usb{it}")
                    usb.append(t)
                    src = ut[:, it * P : (it + 1) * P, :]
                    for a in range(NA):
                        nc.sync.dma_start(
                            t[:, a * KS : (a + 1) * KS, :],
                            src[a * KS : (a + 1) * KS].rearrange("k p o -> p k o"),
                        )

                # ---- winograd-domain conv + inverse transform --------------
                for ot in range(NO):
                    o0 = ot * P
                    for tch in range(TCH):
                        ty0 = tch * TROWS
                        m = mpool.tile([P, NA, NPIX], BF16)
                        for a in range(NA):
                            ps = psum_pool.tile([P, NPIX], F32)
                            j = 0
                            for it in range(NI):
                                for kx in range(KS):
                                    nc.tensor.matmul(
                                        ps[:],
                                        usb[it][:, a * KS + kx, o0 : o0 + P],
                                        vsb[it][:, a, ty0 : ty0 + TROWS, kx : kx + W],
                                        start=(j == 0),
                                        stop=(j == NK - 1),
                                    )
                                    j += 1
                            # demod folded into eviction; cast to bf16
                            nc.scalar.activation(
                                m[:, a, :], ps[:], AF.Copy, scale=dsb[:, ot : ot + 1]
                            )
                        # inverse transform: y0 = m0+m1+m2, y1 = m1-m2-m3
                        t0 = tpool.tile([P, NPIX], BF16, tag="t0")
                        t1 = tpool.tile([P, NPIX], BF16, tag="t1")
                        nc.vector.tensor_tensor(t0[:], m[:, 0, :], m[:, 1, :], OP.add)
                        nc.vector.tensor_tensor(
                            t1[:], m[:, 1, :], m[:, 2, :], OP.subtract
                        )
                        y = ypool.tile([P, TROWS, 2, W], F32)
                        nc.vector.tensor_tensor(
                            y[:, :, 0, :],
                            t0[:].rearrange("p (t x) -> p t x", x=W),
                            m[:, 2, :].rearrange("p (t x) -> p t x", x=W),
                            OP.add,
                        )
                        nc.vector.tensor_tensor(
                            y[:, :, 1, :],
                            t1[:].rearrange("p (t x) -> p t x", x=W),
                            m[:, 3, :].rearrange("p (t x) -> p t x", x=W),
                            OP.subtract,
                        )
                        nc.sync.dma_start(
                            out[o0 : o0 + P, tch * 2 * TROWS : (tch + 1) * 2 * TROWS, :],
                            y[:].rearrange("p t d x -> p (t d) x"),
                        )
    nc.compile()
    return nc


_NC_CACHE = None


def _get_nc():
    global _NC_CACHE
    if _NC_CACHE is None:
        _NC_CACHE = build_nc()
    return _NC_CACHE


def make_in_maps(img, s, weight):
    """Host-side input prep: shard over batch, static weight transforms."""
    img = np.asarray(img, dtype=np.float32)
    s = np.ascontiguousarray(np.asarray(s, dtype=np.float32))
    weight = np.asarray(weight, dtype=np.float32)
    bf = ml_dtypes.bfloat16
    # zero-pad image host-side, cast bf16: [B, NI, P, HP, WP]
    imgp = np.zeros((img.shape[0], NI, P, HP, WP), dtype=bf)
    imgp[:, :, :, 1 : H + 1, 1 : W + 1] = img.reshape(-1, NI, P, H, W)
    # U[a,kx,i,o] = sum_ky G[a,ky] * w[o,i,ky,kx]  -> [12, CIN, COUT] bf16
    G = np.array(
        [[1, 0, 0], [0.5, 0.5, 0.5], [0.5, -0.5, 0.5], [0, 0, 1]], np.float64
    )
    utv = np.einsum("ag,oigx->axio", G, weight.astype(np.float64))
    utv = np.ascontiguousarray(utv.reshape(NK, CIN, COUT)).astype(bf)
    # A_T[i, o] = sum_k weight[o, i, :, :]^2  (static, sample-independent)
    atv = np.ascontiguousarray(
        (weight.astype(np.float64) ** 2).sum(axis=(2, 3)).T
    ).astype(bf)
    return [
        {"img": imgp[b], "s": s[b], "ut": utv, "at": atv} for b in range(N_CORES)
    ]


def kernel(img, s, weight):
    nc = _get_nc()
    in_maps = make_in_maps(img, s, weight)
    res = run_bass_kernel_spmd(nc, in_maps, list(range(N_CORES)))
    return np.stack([res.results[b]["out"] for b in range(N_CORES)], axis=0)


# revision 7
# speedup vs baseline: 1.1416x; 1.1416x over previous
"""Trainium2 Bass kernel for Conv2Demod (StyleGAN2-style modulated conv).

Reference computation (per sample b):
    w[b,o,i,ky,kx] = weight[o,i,ky,kx] * (1 + s[b,i])
    d[b,o]         = rsqrt(sum_{i,ky,kx} w^2 + 1e-8)
    out[b]         = conv2d(img[b], w[b]*d[b,o], pad=1)

v3: 1-D Winograd F(4,3) along y cuts PE work 2x vs direct conv (576 vs
1152 N=512 matmuls/sample; the per-matmul cost on this toolchain is
~270 ns = 512/2.4GHz streaming + a serialized ~56 ns FWL LDWEIGHTS, so
matmul COUNT is the roofline). bf16 operands, fp32 PSUM accumulation.

Per-sample algebra:
  - modulation (1+s[i]) folded into the image (per-partition scale, DVE),
  - demodulation d[o] applied by ACT on the final inverse-transform
    planes (the only PSUM->SBUF copy; no separate m staging -- the 6
    Winograd planes of a chunk live in 6 of the 8 PSUM banks and the
    inverse transform consumes them directly with DVE ops),
  - d[o] from the tiny matvec  A_T[i,o] @ (1+s[i])^2  (A_T host-side),
  - y-transform B^T (points 0,+-1,+-2,inf) factored into 9 tensor_tensor
    + 6 scalar_tensor_tensor DVE ops per i-tile,
  - x-direction stays a direct 3-tap conv via shifted rhs reads,
  - U[a,kx,i,o] = sum_ky G[a,ky] w[o,i,ky,kx] host-precomputed in bf16,
  - inverse transform At=[[1,1,1,1,1,0],[0,1,-1,2,-2,0],[0,1,1,4,4,0],
    [0,1,-1,8,-8,1]] factored into 6 TT + 3 STT + 1 TT ops (fp32),
    final scale+pack via 4 ACT activations into a [ty,dy,x] tile whose
    flat layout equals output row order -> one contiguous DMA per chunk.

Sharding: data-parallel over batch -- 8 samples onto 8 NeuronCores.
"""

import contextlib

import numpy as np
import ml_dtypes

import concourse.bacc as bacc
import concourse.mybir as mybir
import concourse.tile as tile
from concourse.bass_utils import run_bass_kernel_spmd

P = 128          # partitions
CIN = 512
COUT = 512
H = W = 64
KS = 3
MO = 4           # output rows per Winograd tile
NA = MO + KS - 1  # 6 transform planes
NK = NA * KS     # 18 U planes
NI = CIN // P    # 4 i-tiles
NO = COUT // P   # 4 o-tiles
TY = H // MO     # 16 tile-rows
TCH = 2          # tile-row chunks
TROWS = TY // TCH  # 8 tile-rows per chunk
NPIX = TROWS * W   # 512 = matmul N
HP = H + 2
WP = W + 2
EPS = 1e-8
N_CORES = 8

F32 = mybir.dt.float32
BF16 = mybir.dt.bfloat16
AF = mybir.ActivationFunctionType
OP = mybir.AluOpType
_nullcm = contextlib.nullcontext


def build_nc(chain=False, loop_n=None):
    """Per-core program: one sample's modulated conv via 1-D F(4,3)."""
    nc = bacc.Bacc("TRN2", target_bir_lowering=False, debug=False)

    img = nc.dram_tensor("img", [NI, P, HP, WP], BF16, kind="ExternalInput").ap()
    s_in = nc.dram_tensor("s", [CIN], F32, kind="ExternalInput").ap()
    ut = nc.dram_tensor("ut", [NK, CIN, COUT], BF16, kind="ExternalInput").ap()
    at = nc.dram_tensor("at", [CIN, COUT], BF16, kind="ExternalInput").ap()
    out = nc.dram_tensor("out", [COUT, H, W], F32, kind="ExternalOutput").ap()
    s_out = None
    if chain:
        s_out = nc.dram_tensor("s_out", [CIN], F32, kind="ExternalOutput").ap()

    with tile.TileContext(nc) as tc:
        with (
            tc.tile_pool(name="const", bufs=1) as cpool,
            tc.tile_pool(name="persist", bufs=1) as ppool,
            tc.tile_pool(name="imgst", bufs=2) as imgst,
            tc.tile_pool(name="w1", bufs=3) as w1pool,
            tc.tile_pool(name="inv", bufs=12) as invpool,
            tc.tile_pool(name="ypool", bufs=2) as ypool,
            tc.tile_pool(name="psum", bufs=8, space="PSUM") as psum_pool,
        ):
            with (tc.For_i(0, loop_n, 1) if loop_n else _nullcm()):
                # ---- s-derived scalars -------------------------------------
                sraw = cpool.tile([P, NI, 2], F32, tag="sraw")
                for c in range(2):
                    nc.sync.dma_start(
                        sraw[:, :, c], s_in.rearrange("(t p) -> p t", p=P)
                    )
                if chain:
                    nc.sync.dma_start(s_out[:], s_in[:])
                smod = cpool.tile([P, NI, 2], F32, tag="smod")  # 1 + s
                nc.scalar.activation(smod[:], sraw[:], AF.Copy, bias=1.0)
                tsq = cpool.tile([P, NI, 2], BF16, tag="tsq")   # (1 + s)^2
                nc.scalar.square(tsq[:], smod[:])

                # ---- demod d[o] = 1/sqrt(A_T.T @ tsq + eps) ----------------
                at_sb = ppool.tile([P, NI, COUT], BF16, tag="at_sb")
                nc.sync.dma_start(at_sb[:], at.rearrange("(t p) o -> p t o", p=P))
                dsb = cpool.tile([P, NO], F32, tag="dsb")
                dtmp = cpool.tile([P, NO], F32, tag="dtmp")
                epst = cpool.tile([P, 1], F32, tag="epst")
                nc.vector.memset(epst[:], EPS)
                for ot in range(NO):
                    o0 = ot * P
                    psd = psum_pool.tile([P, 2], F32, name="psd", tag="ps")
                    for it in range(NI):
                        nc.tensor.matmul(
                            psd[:],
                            at_sb[:, it, o0 : o0 + P],
                            tsq[:, it, :],
                            start=(it == 0),
                            stop=(it == NI - 1),
                        )
                    nc.scalar.activation(
                        dtmp[:, ot : ot + 1], psd[:, 0:1], AF.Sqrt, bias=epst[:]
                    )
                nc.vector.reciprocal(dsb[:], dtmp[:])

                # ---- modulated y-transformed image V[a] --------------------
                # B^T rows (points 0,1,-1,2,-2,inf):
                #   r0 = 4(d0-d2) - (d2-d4)      r3 =  2(d3-d1) + (d4-d2)
                #   r1 = -4(d1+d2) + (d3+d4)     r4 = -2(d3-d1) + (d4-d2)
                #   r2 = 4(d1-d2) - (d3-d4)      r5 = -4(d3-d1) - (d3-d5)
                vsb = []
                for it in range(NI):
                    t = ppool.tile([P, NA, TY, WP], BF16, tag=f"vsb{it}")
                    vsb.append(t)
                for it in range(NI):
                    ir = imgst.tile([P, HP, WP], BF16, name="ir", tag="ir")
                    nc.sync.dma_start(ir[:], img[it])
                    nc.vector.tensor_scalar_mul(ir[:], ir[:], smod[:, it, 0:1])
                    dd = [ir[:, k : k + 4 * (TY - 1) + 1 : 4, :] for k in range(NA)]
                    v = vsb[it]

                    def w1(nm):
                        return w1pool.tile([P, TY, WP], BF16, name=nm, tag="w1")

                    u_ = w1("u_")
                    nc.vector.tensor_tensor(u_[:], dd[0], dd[2], OP.subtract)
                    v_ = w1("v_")
                    nc.vector.tensor_tensor(v_[:], dd[2], dd[4], OP.subtract)
                    nc.vector.scalar_tensor_tensor(
                        v[:, 0], u_[:], 4.0, v_[:], OP.mult, OP.subtract
                    )
                    p_ = w1("p_")
                    nc.vector.tensor_tensor(p_[:], dd[1], dd[2], OP.add)
                    q_ = w1("q_")
                    nc.vector.tensor_tensor(q_[:], dd[3], dd[4], OP.add)
                    nc.vector.scalar_tensor_tensor(
                        v[:, 1], p_[:], -4.0, q_[:], OP.mult, OP.add
                    )
                    e_ = w1("e_")
                    nc.vector.tensor_tensor(e_[:], dd[1], dd[2], OP.subtract)
                    f_ = w1("f_")
                    nc.vector.tensor_tensor(f_[:], dd[3], dd[4], OP.subtract)
                    nc.vector.scalar_tensor_tensor(
                        v[:, 2], e_[:], 4.0, f_[:], OP.mult, OP.subtract
                    )
                    g_ = w1("g_")
                    nc.vector.tensor_tensor(g_[:], dd[3], dd[1], OP.subtract)
                    h_ = w1("h_")
                    nc.vector.tensor_tensor(h_[:], dd[4], dd[2], OP.subtract)
                    z_ = w1("z_")
                    nc.vector.tensor_tensor(z_[:], dd[3], dd[5], OP.subtract)
                    nc.vector.scalar_tensor_tensor(
                        v[:, 3], g_[:], 2.0, h_[:], OP.mult, OP.add
                    )
                    nc.vector.scalar_tensor_tensor(
                        v[:, 4], g_[:], -2.0, h_[:], OP.mult, OP.add
                    )
                    nc.vector.scalar_tensor_tensor(
                        v[:, 5], g_[:], -4.0, z_[:], OP.mult, OP.subtract
                    )

                # ---- transformed weights U ---------------------------------
                usb = []
                for it in range(NI):
                    t = ppool.tile([P, NK, COUT], BF16, tag=f"usb{it}")
                    usb.append(t)
                    src = ut[:, it * P : (it + 1) * P, :]
                    for a in range(NA):
                        nc.sync.dma_start(
                            t[:, a * KS : (a + 1) * KS, :],
                            src[a * KS : (a + 1) * KS].rearrange("k p o -> p k o"),
                        )

                # ---- winograd-domain conv + inverse transform --------------
                # emit a-groups in consumption order so PSUM banks free early
                A_ORDER = (1, 2, 3, 4, 0, 5)
                for ot in range(NO):
                    o0 = ot * P
                    for tch in range(TCH):
                        ty0 = tch * TROWS
                        ps = {}
                        for a in A_ORDER:
                            p_ps = psum_pool.tile(
                                [P, NPIX], F32, name=f"ps{a}", tag="ps"
                            )
                            ps[a] = p_ps
                            j = 0
                            for it in range(NI):
                                for kx in range(KS):
                                    nc.tensor.matmul(
                                        p_ps[:],
                                        usb[it][:, a * KS + kx, o0 : o0 + P],
                                        vsb[it][:, a, ty0 : ty0 + TROWS, kx : kx + W],
                                        start=(j == 0),
                                        stop=(j == NK - 1),
                                    )
                                    j += 1

                        def iv(nm):
                            return invpool.tile([P, NPIX], F32, name=nm, tag="iv")

                        # inverse transform in fp32 from PSUM. DVE may read
                        # only ONE PSUM operand per op, so stage m1/m3 to
                        # SBUF via ACT copies first.
                        e1 = iv("e1")
                        nc.scalar.activation(e1[:], ps[1][:], AF.Copy)
                        e3 = iv("e3")
                        nc.scalar.activation(e3[:], ps[3][:], AF.Copy)
                        a1 = iv("a1")
                        nc.vector.tensor_tensor(a1[:], e1[:], ps[2][:], OP.subtract)
                        c1 = iv("c1")
                        nc.vector.tensor_tensor(c1[:], e1[:], ps[2][:], OP.add)
                        b1 = iv("b1")
                        nc.vector.tensor_tensor(b1[:], e3[:], ps[4][:], OP.subtract)
                        d1 = iv("d1")
                        nc.vector.tensor_tensor(d1[:], e3[:], ps[4][:], OP.add)
                        t_ = iv("t_")
                        nc.vector.tensor_tensor(t_[:], ps[0][:], c1[:], OP.add)
                        tt = iv("tt")
                        nc.vector.tensor_tensor(tt[:], t_[:], d1[:], OP.add)
                        y1r = iv("y1r")
                        nc.vector.scalar_tensor_tensor(
                            y1r[:], b1[:], 2.0, a1[:], OP.mult, OP.add
                        )
                        y2r = iv("y2r")
                        nc.vector.scalar_tensor_tensor(
                            y2r[:], d1[:], 4.0, c1[:], OP.mult, OP.add
                        )
                        y3r = iv("y3r")
                        nc.vector.scalar_tensor_tensor(
                            y3r[:], b1[:], 8.0, a1[:], OP.mult, OP.add
                        )
                        y3 = iv("y3")
                        nc.vector.tensor_tensor(y3[:], y3r[:], ps[5][:], OP.add)

                        # final demod scale + pack [ty, dy, x] (ACT)
                        y = ypool.tile([P, TROWS, MO, W], F32, name="y", tag="y")
                        for dy, src_t in enumerate((tt, y1r, y2r, y3)):
                            nc.scalar.activation(
                                y[:, :, dy, :],
                                src_t[:].rearrange("p (t x) -> p t x", x=W),
                                AF.Copy,
                                scale=dsb[:, ot : ot + 1],
                            )
                        nc.sync.dma_start(
                            out[o0 : o0 + P, tch * MO * TROWS : (tch + 1) * MO * TROWS, :],
                            y[:].rearrange("p t d x -> p (t d) x"),
                        )
    nc.compile()
    return nc


_NC_CACHE = None


def _get_nc():
    global _NC_CACHE
    if _NC_CACHE is None:
        _NC_CACHE = build_nc()
    return _NC_CACHE


def make_in_maps(img, s, weight):
    """Host-side input prep: shard over batch, static weight transforms."""
    img = np.asarray(img, dtype=np.float32)
    s = np.ascontiguousarray(np.asarray(s, dtype=np.float32))
    weight = np.asarray(weight, dtype=np.float32)
    bf = ml_dtypes.bfloat16
    imgp = np.zeros((img.shape[0], NI, P, HP, WP), dtype=bf)
    imgp[:, :, :, 1 : H + 1, 1 : W + 1] = img.reshape(-1, NI, P, H, W)
    # F(4,3) G (points 0,1,-1,2,-2,inf)
    G = np.array(
        [
            [1 / 4, 0, 0],
            [-1 / 6, -1 / 6, -1 / 6],
            [-1 / 6, 1 / 6, -1 / 6],
            [1 / 24, 1 / 12, 1 / 6],
            [1 / 24, -1 / 12, 1 / 6],
            [0, 0, 1],
        ],
        np.float64,
    )
    utv = np.einsum("ag,oigx->axio", G, weight.astype(np.float64))
    utv = np.ascontiguousarray(utv.reshape(NK, CIN, COUT)).astype(bf)
    atv = np.ascontiguousarray(
        (weight.astype(np.float64) ** 2).sum(axis=(2, 3)).T
    ).astype(bf)
    return [
        {"img": imgp[b], "s": s[b], "ut": utv, "at": atv} for b in range(N_CORES)
    ]


def kernel(img, s, weight):
    nc = _get_nc()
    in_maps = make_in_maps(img, s, weight)
    res = run_bass_kernel_spmd(nc, in_maps, list(range(N_CORES)))
    return np.stack([res.results[b]["out"] for b in range(N_CORES)], axis=0)


# revision 10
# speedup vs baseline: 1.1713x; 1.0260x over previous
"""Trainium2 Bass kernel for Conv2Demod (StyleGAN2-style modulated conv).

Reference computation (per sample b):
    w[b,o,i,ky,kx] = weight[o,i,ky,kx] * (1 + s[b,i])
    d[b,o]         = rsqrt(sum_{i,ky,kx} w^2 + 1e-8)
    out[b]         = conv2d(img[b], w[b]*d[b,o], pad=1)

v3: 1-D Winograd F(4,3) along y cuts PE work 2x vs direct conv (576 vs
1152 N=512 matmuls/sample; the per-matmul cost on this toolchain is
~270 ns = 512/2.4GHz streaming + a serialized ~56 ns FWL LDWEIGHTS, so
matmul COUNT is the roofline). bf16 operands, fp32 PSUM accumulation.

Per-sample algebra:
  - modulation (1+s[i]) folded into the image (per-partition scale, DVE),
  - demodulation d[o] applied by ACT on the final inverse-transform
    planes (the only PSUM->SBUF copy; no separate m staging -- the 6
    Winograd planes of a chunk live in 6 of the 8 PSUM banks and the
    inverse transform consumes them directly with DVE ops),
  - d[o] from the tiny matvec  A_T[i,o] @ (1+s[i])^2  (A_T host-side),
  - y-transform B^T (points 0,+-1,+-2,inf) factored into 9 tensor_tensor
    + 6 scalar_tensor_tensor DVE ops per i-tile,
  - x-direction stays a direct 3-tap conv via shifted rhs reads,
  - U[a,kx,i,o] = sum_ky G[a,ky] w[o,i,ky,kx] host-precomputed in bf16,
  - inverse transform At=[[1,1,1,1,1,0],[0,1,-1,2,-2,0],[0,1,1,4,4,0],
    [0,1,-1,8,-8,1]] factored into 6 TT + 3 STT + 1 TT ops (fp32),
    final scale+pack via 4 ACT activations into a [ty,dy,x] tile whose
    flat layout equals output row order -> one contiguous DMA per chunk.

Sharding: data-parallel over batch -- 8 samples onto 8 NeuronCores.
"""

import contextlib

import numpy as np
import ml_dtypes

import concourse.bacc as bacc
import concourse.mybir as mybir
import concourse.tile as tile
from concourse.bass_utils import run_bass_kernel_spmd

P = 128          # partitions
CIN = 512
COUT = 512
H = W = 64
KS = 3
MO = 4           # output rows per Winograd tile
NA = MO + KS - 1  # 6 transform planes
NK = NA * KS     # 18 U planes
NI = CIN // P    # 4 i-tiles
NO = COUT // P   # 4 o-tiles
TY = H // MO     # 16 tile-rows
TCH = 2          # tile-row chunks
TROWS = TY // TCH  # 8 tile-rows per chunk
NPIX = TROWS * W   # 512 = matmul N
HP = H + 2
WP = W + 2
EPS = 1e-8
N_CORES = 8

F32 = mybir.dt.float32
BF16 = mybir.dt.bfloat16
AF = mybir.ActivationFunctionType
OP = mybir.AluOpType
_nullcm = contextlib.nullcontext


def build_nc(chain=False, loop_n=None, staggered=False):
    """Per-core program: one sample's modulated conv via 1-D F(4,3)."""
    nc = bacc.Bacc("TRN2", target_bir_lowering=False, debug=False)

    img = nc.dram_tensor("img", [NI, P, HP, WP], BF16, kind="ExternalInput").ap()
    s_in = nc.dram_tensor("s", [CIN], F32, kind="ExternalInput").ap()
    ut = nc.dram_tensor("ut", [NK, CIN, COUT], BF16, kind="ExternalInput").ap()
    at = nc.dram_tensor("at", [CIN, COUT], BF16, kind="ExternalInput").ap()
    out = nc.dram_tensor("out", [COUT, H, W], F32, kind="ExternalOutput").ap()
    s_out = None
    if chain:
        s_out = nc.dram_tensor("s_out", [CIN], F32, kind="ExternalOutput").ap()

    with tile.TileContext(nc) as tc:
        with (
            tc.tile_pool(name="const", bufs=1) as cpool,
            tc.tile_pool(name="persist", bufs=1) as ppool,
            tc.tile_pool(name="imgst", bufs=2) as imgst,
            tc.tile_pool(name="w1", bufs=3) as w1pool,
            tc.tile_pool(name="inv", bufs=12) as invpool,
            tc.tile_pool(name="ypool", bufs=2) as ypool,
            tc.tile_pool(name="psum", bufs=8, space="PSUM") as psum_pool,
        ):
            loop_kw = dict(
                hint_engines=(
                    mybir.EngineType.PE,
                    mybir.EngineType.DVE,
                    mybir.EngineType.Activation,
                ),
            )
            if staggered:
                loop_kw["staggered_reset"] = True
            with (tc.For_i(0, loop_n, 1, **loop_kw) if loop_n else _nullcm()):
                # ---- s-derived scalars -------------------------------------
                sraw = cpool.tile([P, NI, 2], F32, tag="sraw")
                for c in range(2):
                    nc.sync.dma_start(
                        sraw[:, :, c], s_in.rearrange("(t p) -> p t", p=P)
                    )
                if chain:
                    nc.sync.dma_start(s_out[:], s_in[:])
                smod = cpool.tile([P, NI, 2], F32, tag="smod")  # 1 + s
                nc.scalar.activation(smod[:], sraw[:], AF.Copy, bias=1.0)
                tsq = cpool.tile([P, NI, 2], BF16, tag="tsq")   # (1 + s)^2
                nc.scalar.square(tsq[:], smod[:])

                # ---- demod d[o] = 1/sqrt(A_T.T @ tsq + eps) ----------------
                at_sb = ppool.tile([P, NI, COUT], BF16, tag="at_sb")
                nc.sync.dma_start(at_sb[:], at.rearrange("(t p) o -> p t o", p=P))
                dsb = cpool.tile([P, NO], F32, tag="dsb")
                dtmp = cpool.tile([P, NO], F32, tag="dtmp")
                epst = cpool.tile([P, 1], F32, tag="epst")
                nc.vector.memset(epst[:], EPS)
                for ot in range(NO):
                    o0 = ot * P
                    psd = psum_pool.tile([P, 2], F32, name="psd", tag="ps")
                    for it in range(NI):
                        nc.tensor.matmul(
                            psd[:],
                            at_sb[:, it, o0 : o0 + P],
                            tsq[:, it, :],
                            start=(it == 0),
                            stop=(it == NI - 1),
                        )
                    nc.scalar.activation(
                        dtmp[:, ot : ot + 1], psd[:, 0:1], AF.Sqrt, bias=epst[:]
                    )
                nc.vector.reciprocal(dsb[:], dtmp[:])

                # ---- modulated y-transformed image V[a] --------------------
                # B^T rows (points 0,1,-1,2,-2,inf):
                #   r0 = 4(d0-d2) - (d2-d4)      r3 =  2(d3-d1) + (d4-d2)
                #   r1 = -4(d1+d2) + (d3+d4)     r4 = -2(d3-d1) + (d4-d2)
                #   r2 = 4(d1-d2) - (d3-d4)      r5 = -4(d3-d1) - (d3-d5)
                vsb = []
                for it in range(NI):
                    t = ppool.tile([P, NA, TY, WP], BF16, tag=f"vsb{it}")
                    vsb.append(t)
                for it in range(NI):
                    ir = imgst.tile([P, HP, WP], BF16, name="ir", tag="ir")
                    nc.sync.dma_start(ir[:], img[it])
                    nc.vector.tensor_scalar_mul(ir[:], ir[:], smod[:, it, 0:1])
                    dd = [ir[:, k : k + 4 * (TY - 1) + 1 : 4, :] for k in range(NA)]
                    v = vsb[it]

                    def w1(nm):
                        return w1pool.tile([P, TY, WP], BF16, name=nm, tag="w1")

                    # emit planes in A_ORDER consumption order (a=1 first);
                    # note h = d4 - d2 = -(d2 - d4) = -v_, so r3/r4 reuse v_.
                    p_ = w1("p_")
                    nc.vector.tensor_tensor(p_[:], dd[1], dd[2], OP.add)
                    q_ = w1("q_")
                    nc.vector.tensor_tensor(q_[:], dd[3], dd[4], OP.add)
                    nc.vector.scalar_tensor_tensor(
                        v[:, 1], p_[:], -4.0, q_[:], OP.mult, OP.add
                    )
                    e_ = w1("e_")
                    nc.vector.tensor_tensor(e_[:], dd[1], dd[2], OP.subtract)
                    f_ = w1("f_")
                    nc.vector.tensor_tensor(f_[:], dd[3], dd[4], OP.subtract)
                    nc.vector.scalar_tensor_tensor(
                        v[:, 2], e_[:], 4.0, f_[:], OP.mult, OP.subtract
                    )
                    g_ = w1("g_")
                    nc.vector.tensor_tensor(g_[:], dd[3], dd[1], OP.subtract)
                    v_ = w1("v_")
                    nc.vector.tensor_tensor(v_[:], dd[2], dd[4], OP.subtract)
                    nc.vector.scalar_tensor_tensor(
                        v[:, 3], g_[:], 2.0, v_[:], OP.mult, OP.subtract
                    )
                    nc.vector.scalar_tensor_tensor(
                        v[:, 4], g_[:], -2.0, v_[:], OP.mult, OP.subtract
                    )
                    u_ = w1("u_")
                    nc.vector.tensor_tensor(u_[:], dd[0], dd[2], OP.subtract)
                    nc.vector.scalar_tensor_tensor(
                        v[:, 0], u_[:], 4.0, v_[:], OP.mult, OP.subtract
                    )
                    z_ = w1("z_")
                    nc.vector.tensor_tensor(z_[:], dd[3], dd[5], OP.subtract)
                    nc.vector.scalar_tensor_tensor(
                        v[:, 5], g_[:], -4.0, z_[:], OP.mult, OP.subtract
                    )

                # ---- transformed weights U ---------------------------------
                usb = []
                for it in range(NI):
                    t = ppool.tile([P, NK, COUT], BF16, tag=f"usb{it}")
                    usb.append(t)
                    src = ut[:, it * P : (it + 1) * P, :]
                    for a in range(NA):
                        nc.sync.dma_start(
                            t[:, a * KS : (a + 1) * KS, :],
                            src[a * KS : (a + 1) * KS].rearrange("k p o -> p k o"),
                        )

                # ---- winograd-domain conv + inverse transform --------------
                # emit a-groups in consumption order so PSUM banks free early
                A_ORDER = (1, 2, 3, 4, 0, 5)
                for ot in range(NO):
                    o0 = ot * P
                    for tch in range(TCH):
                        ty0 = tch * TROWS
                        ps = {}
                        for a in A_ORDER:
                            p_ps = psum_pool.tile(
                                [P, NPIX], F32, name=f"ps{a}", tag="ps"
                            )
                            ps[a] = p_ps
                            j = 0
                            for it in range(NI):
                                for kx in range(KS):
                                    nc.tensor.matmul(
                                        p_ps[:],
                                        usb[it][:, a * KS + kx, o0 : o0 + P],
                                        vsb[it][:, a, ty0 : ty0 + TROWS, kx : kx + W],
                                        start=(j == 0),
                                        stop=(j == NK - 1),
                                    )
                                    j += 1

                        def iv(nm):
                            return invpool.tile([P, NPIX], F32, name=nm, tag="iv")

                        # inverse transform in fp32 from PSUM. DVE may read
                        # only ONE PSUM operand per op, so stage m1/m3 to
                        # SBUF via ACT copies first.
                        e1 = iv("e1")
                        nc.scalar.activation(e1[:], ps[1][:], AF.Copy)
                        e3 = iv("e3")
                        nc.scalar.activation(e3[:], ps[3][:], AF.Copy)
                        a1 = iv("a1")
                        nc.vector.tensor_tensor(a1[:], e1[:], ps[2][:], OP.subtract)
                        c1 = iv("c1")
                        nc.vector.tensor_tensor(c1[:], e1[:], ps[2][:], OP.add)
                        b1 = iv("b1")
                        nc.vector.tensor_tensor(b1[:], e3[:], ps[4][:], OP.subtract)
                        d1 = iv("d1")
                        nc.vector.tensor_tensor(d1[:], e3[:], ps[4][:], OP.add)
                        t_ = iv("t_")
                        nc.vector.tensor_tensor(t_[:], ps[0][:], c1[:], OP.add)
                        tt = iv("tt")
                        nc.vector.tensor_tensor(tt[:], t_[:], d1[:], OP.add)
                        y1r = iv("y1r")
                        nc.vector.scalar_tensor_tensor(
                            y1r[:], b1[:], 2.0, a1[:], OP.mult, OP.add
                        )
                        y2r = iv("y2r")
                        nc.vector.scalar_tensor_tensor(
                            y2r[:], d1[:], 4.0, c1[:], OP.mult, OP.add
                        )
                        y3r = iv("y3r")
                        nc.vector.scalar_tensor_tensor(
                            y3r[:], b1[:], 8.0, a1[:], OP.mult, OP.add
                        )
                        y3 = iv("y3")
                        nc.vector.tensor_tensor(y3[:], y3r[:], ps[5][:], OP.add)

                        # final demod scale + pack [ty, dy, x] (ACT)
                        y = ypool.tile([P, TROWS, MO, W], F32, name="y", tag="y")
                        for dy, src_t in enumerate((tt, y1r, y2r, y3)):
                            nc.scalar.activation(
                                y[:, :, dy, :],
                                src_t[:].rearrange("p (t x) -> p t x", x=W),
                                AF.Copy,
                                scale=dsb[:, ot : ot + 1],
                            )
                        nc.sync.dma_start(
                            out[o0 : o0 + P, tch * MO * TROWS : (tch + 1) * MO * TROWS, :],
                            y[:].rearrange("p t d x -> p (t d) x"),
                        )
    nc.compile()
    return nc


_NC_CACHE = None


def _get_nc():
    global _NC_CACHE
    if _NC_CACHE is None:
        _NC_CACHE = build_nc()
    return _NC_CACHE


def make_in_maps(img, s, weight):
    """Host-side input prep: shard over batch, static weight transforms."""
    img = np.asarray(img, dtype=np.float32)
    s = np.ascontiguousarray(np.asarray(s, dtype=np.float32))
    weight = np.asarray(weight, dtype=np.float32)
    bf = ml_dtypes.bfloat16
    imgp = np.zeros((img.shape[0], NI, P, HP, WP), dtype=bf)
    imgp[:, :, :, 1 : H + 1, 1 : W + 1] = img.reshape(-1, NI, P, H, W)
    # F(4,3) G (points 0,1,-1,2,-2,inf)
    G = np.array(
        [
            [1 / 4, 0, 0],
            [-1 / 6, -1 / 6, -1 / 6],
            [-1 / 6, 1 / 6, -1 / 6],
            [1 / 24, 1 / 12, 1 / 6],
            [1 / 24, -1 / 12, 1 / 6],
            [0, 0, 1],
        ],
        np.float64,
    )
    utv = np.einsum("ag,oigx->axio", G, weight.astype(np.float64))
    utv = np.ascontiguousarray(utv.reshape(NK, CIN, COUT)).astype(bf)
    atv = np.ascontiguousarray(
        (weight.astype(np.float64) ** 2).sum(axis=(2, 3)).T
    ).astype(bf)
    return [
        {"img": imgp[b], "s": s[b], "ut": utv, "at": atv} for b in range(N_CORES)
    ]


def kernel(img, s, weight):
    nc = _get_nc()
    in_maps = make_in_maps(img, s, weight)
    res = run_bass_kernel_spmd(nc, in_maps, list(range(N_CORES)))
    return np.stack([res.results[b]["out"] for b in range(N_CORES)], axis=0)


# revision 11
# speedup vs baseline: 1.1836x; 1.0105x over previous
"""Trainium2 Bass kernel for Conv2Demod (StyleGAN2-style modulated conv).

Reference computation (per sample b):
    w[b,o,i,ky,kx] = weight[o,i,ky,kx] * (1 + s[b,i])
    d[b,o]         = rsqrt(sum_{i,ky,kx} w^2 + 1e-8)
    out[b]         = conv2d(img[b], w[b]*d[b,o], pad=1)

v3: 1-D Winograd F(4,3) along y cuts PE work 2x vs direct conv (576 vs
1152 N=512 matmuls/sample; the per-matmul cost on this toolchain is
~270 ns = 512/2.4GHz streaming + a serialized ~56 ns FWL LDWEIGHTS, so
matmul COUNT is the roofline). bf16 operands, fp32 PSUM accumulation.

Per-sample algebra:
  - modulation (1+s[i]) folded into the image (per-partition scale, DVE),
  - demodulation d[o] applied by ACT on the final inverse-transform
    planes (the only PSUM->SBUF copy; no separate m staging -- the 6
    Winograd planes of a chunk live in 6 of the 8 PSUM banks and the
    inverse transform consumes them directly with DVE ops),
  - d[o] from the tiny matvec  A_T[i,o] @ (1+s[i])^2  (A_T host-side),
  - y-transform B^T (points 0,+-1,+-2,inf) factored into 9 tensor_tensor
    + 6 scalar_tensor_tensor DVE ops per i-tile,
  - x-direction stays a direct 3-tap conv via shifted rhs reads,
  - U[a,kx,i,o] = sum_ky G[a,ky] w[o,i,ky,kx] host-precomputed in bf16,
  - inverse transform At=[[1,1,1,1,1,0],[0,1,-1,2,-2,0],[0,1,1,4,4,0],
    [0,1,-1,8,-8,1]] factored into 6 TT + 3 STT + 1 TT ops (fp32),
    final scale+pack via 4 ACT activations into a [ty,dy,x] tile whose
    flat layout equals output row order -> one contiguous DMA per chunk.

Sharding: data-parallel over batch -- 8 samples onto 8 NeuronCores.
"""

import contextlib

import numpy as np
import ml_dtypes

import concourse.bacc as bacc
import concourse.mybir as mybir
import concourse.tile as tile
from concourse.bass_utils import run_bass_kernel_spmd

P = 128          # partitions
CIN = 512
COUT = 512
H = W = 64
KS = 3
MO = 4           # output rows per Winograd tile
NA = MO + KS - 1  # 6 transform planes
NK = NA * KS     # 18 U planes
NI = CIN // P    # 4 i-tiles
NO = COUT // P   # 4 o-tiles
TY = H // MO     # 16 tile-rows
TCH = 2          # tile-row chunks
TROWS = TY // TCH  # 8 tile-rows per chunk
NPIX = TROWS * W   # 512 = matmul N
HP = H + 2
WP = W + 2
EPS = 1e-8
N_CORES = 8

F32 = mybir.dt.float32
BF16 = mybir.dt.bfloat16
AF = mybir.ActivationFunctionType
OP = mybir.AluOpType
_nullcm = contextlib.nullcontext


def build_nc(chain=False, loop_n=None, staggered=False):
    """Per-core program: one sample's modulated conv via 1-D F(4,3)."""
    nc = bacc.Bacc("TRN2", target_bir_lowering=False, debug=False)

    img = nc.dram_tensor("img", [NI, P, HP, WP], BF16, kind="ExternalInput").ap()
    s_in = nc.dram_tensor("s", [CIN], F32, kind="ExternalInput").ap()
    ut = nc.dram_tensor("ut", [NK, CIN, COUT], BF16, kind="ExternalInput").ap()
    at = nc.dram_tensor("at", [CIN, COUT], BF16, kind="ExternalInput").ap()
    out = nc.dram_tensor("out", [COUT, H, W], F32, kind="ExternalOutput").ap()
    s_out = None
    if chain:
        s_out = nc.dram_tensor("s_out", [CIN], F32, kind="ExternalOutput").ap()

    with tile.TileContext(nc) as tc:
        with (
            tc.tile_pool(name="const", bufs=1) as cpool,
            tc.tile_pool(name="persist", bufs=1) as ppool,
            tc.tile_pool(name="imgst", bufs=2) as imgst,
            tc.tile_pool(name="w1", bufs=3) as w1pool,
            tc.tile_pool(name="inv", bufs=12) as invpool,
            tc.tile_pool(name="ypool", bufs=2) as ypool,
            tc.tile_pool(name="psum", bufs=8, space="PSUM") as psum_pool,
        ):
            loop_kw = dict(
                hint_engines=(
                    mybir.EngineType.PE,
                    mybir.EngineType.DVE,
                    mybir.EngineType.Activation,
                ),
            )
            if staggered:
                loop_kw["staggered_reset"] = True
            with (tc.For_i(0, loop_n, 1, **loop_kw) if loop_n else _nullcm()):
                # ---- s-derived scalars -------------------------------------
                sraw = cpool.tile([P, NI, 2], F32, tag="sraw")
                for c in range(2):
                    nc.sync.dma_start(
                        sraw[:, :, c], s_in.rearrange("(t p) -> p t", p=P)
                    )
                if chain:
                    nc.sync.dma_start(s_out[:], s_in[:])
                smod = cpool.tile([P, NI, 2], F32, tag="smod")  # 1 + s
                nc.scalar.activation(smod[:], sraw[:], AF.Copy, bias=1.0)
                tsq = cpool.tile([P, NI, 2], BF16, tag="tsq")   # (1 + s)^2
                nc.scalar.square(tsq[:], smod[:])

                # ---- demod d[o] = 1/sqrt(A_T.T @ tsq + eps) ----------------
                at_sb = ppool.tile([P, NI, COUT], BF16, tag="at_sb")
                nc.sync.dma_start(at_sb[:], at.rearrange("(t p) o -> p t o", p=P))
                dsb = cpool.tile([P, NO], F32, tag="dsb")
                dtmp = cpool.tile([P, NO], F32, tag="dtmp")
                epst = cpool.tile([P, 1], F32, tag="epst")
                nc.vector.memset(epst[:], EPS)
                for ot in range(NO):
                    o0 = ot * P
                    psd = psum_pool.tile([P, 2], F32, name="psd", tag="ps")
                    for it in range(NI):
                        nc.tensor.matmul(
                            psd[:],
                            at_sb[:, it, o0 : o0 + P],
                            tsq[:, it, :],
                            start=(it == 0),
                            stop=(it == NI - 1),
                        )
                    nc.scalar.activation(
                        dtmp[:, ot : ot + 1], psd[:, 0:1], AF.Sqrt, bias=epst[:]
                    )
                nc.vector.reciprocal(dsb[:], dtmp[:])

                # ---- modulated y-transformed image V[a] --------------------
                # B^T rows (points 0,1,-1,2,-2,inf):
                #   r0 = 4(d0-d2) - (d2-d4)      r3 =  2(d3-d1) + (d4-d2)
                #   r1 = -4(d1+d2) + (d3+d4)     r4 = -2(d3-d1) + (d4-d2)
                #   r2 = 4(d1-d2) - (d3-d4)      r5 = -4(d3-d1) - (d3-d5)
                vsb = []
                for it in range(NI):
                    t = ppool.tile([P, NA, TY, WP], BF16, tag=f"vsb{it}")
                    vsb.append(t)
                for it in range(NI):
                    ir = imgst.tile([P, HP, WP], BF16, name="ir", tag="ir")
                    nc.sync.dma_start(ir[:], img[it])
                    # modulation scale on ACT -- keeps the DVE stage1 chain
                    # (the iteration's critical path) as short as possible
                    nc.scalar.activation(
                        ir[:], ir[:], AF.Copy, scale=smod[:, it, 0:1]
                    )
                    dd = [ir[:, k : k + 4 * (TY - 1) + 1 : 4, :] for k in range(NA)]
                    v = vsb[it]

                    def w1(nm):
                        return w1pool.tile([P, TY, WP], BF16, name=nm, tag="w1")

                    # emit planes in A_ORDER consumption order (a=1 first);
                    # note h = d4 - d2 = -(d2 - d4) = -v_, so r3/r4 reuse v_.
                    p_ = w1("p_")
                    nc.vector.tensor_tensor(p_[:], dd[1], dd[2], OP.add)
                    q_ = w1("q_")
                    nc.vector.tensor_tensor(q_[:], dd[3], dd[4], OP.add)
                    nc.vector.scalar_tensor_tensor(
                        v[:, 1], p_[:], -4.0, q_[:], OP.mult, OP.add
                    )
                    e_ = w1("e_")
                    nc.vector.tensor_tensor(e_[:], dd[1], dd[2], OP.subtract)
                    f_ = w1("f_")
                    nc.vector.tensor_tensor(f_[:], dd[3], dd[4], OP.subtract)
                    nc.vector.scalar_tensor_tensor(
                        v[:, 2], e_[:], 4.0, f_[:], OP.mult, OP.subtract
                    )
                    g_ = w1("g_")
                    nc.vector.tensor_tensor(g_[:], dd[3], dd[1], OP.subtract)
                    v_ = w1("v_")
                    nc.vector.tensor_tensor(v_[:], dd[2], dd[4], OP.subtract)
                    nc.vector.scalar_tensor_tensor(
                        v[:, 3], g_[:], 2.0, v_[:], OP.mult, OP.subtract
                    )
                    nc.vector.scalar_tensor_tensor(
                        v[:, 4], g_[:], -2.0, v_[:], OP.mult, OP.subtract
                    )
                    u_ = w1("u_")
                    nc.vector.tensor_tensor(u_[:], dd[0], dd[2], OP.subtract)
                    nc.vector.scalar_tensor_tensor(
                        v[:, 0], u_[:], 4.0, v_[:], OP.mult, OP.subtract
                    )
                    z_ = w1("z_")
                    nc.vector.tensor_tensor(z_[:], dd[3], dd[5], OP.subtract)
                    nc.vector.scalar_tensor_tensor(
                        v[:, 5], g_[:], -4.0, z_[:], OP.mult, OP.subtract
                    )

                # ---- transformed weights U ---------------------------------
                usb = []
                for it in range(NI):
                    t = ppool.tile([P, NK, COUT], BF16, tag=f"usb{it}")
                    usb.append(t)
                    src = ut[:, it * P : (it + 1) * P, :]
                    for a in range(NA):
                        nc.sync.dma_start(
                            t[:, a * KS : (a + 1) * KS, :],
                            src[a * KS : (a + 1) * KS].rearrange("k p o -> p k o"),
                        )

                # ---- winograd-domain conv + inverse transform --------------
                # emit a-groups in consumption order so PSUM banks free early
                A_ORDER = (1, 2, 3, 4, 0, 5)
                for ot in range(NO):
                    o0 = ot * P
                    for tch in range(TCH):
                        ty0 = tch * TROWS
                        ps = {}
                        for a in A_ORDER:
                            p_ps = psum_pool.tile(
                                [P, NPIX], F32, name=f"ps{a}", tag="ps"
                            )
                            ps[a] = p_ps
                            j = 0
                            for it in range(NI):
                                for kx in range(KS):
                                    nc.tensor.matmul(
                                        p_ps[:],
                                        usb[it][:, a * KS + kx, o0 : o0 + P],
                                        vsb[it][:, a, ty0 : ty0 + TROWS, kx : kx + W],
                                        start=(j == 0),
                                        stop=(j == NK - 1),
                                    )
                                    j += 1

                        def iv(nm):
                            return invpool.tile([P, NPIX], F32, name=nm, tag="iv")

                        # inverse transform in fp32 from PSUM. DVE may read
                        # only ONE PSUM operand per op, so stage m1/m3 to
                        # SBUF via ACT copies first.
                        e1 = iv("e1")
                        nc.scalar.activation(e1[:], ps[1][:], AF.Copy)
                        e3 = iv("e3")
                        nc.scalar.activation(e3[:], ps[3][:], AF.Copy)
                        a1 = iv("a1")
                        nc.vector.tensor_tensor(a1[:], e1[:], ps[2][:], OP.subtract)
                        c1 = iv("c1")
                        nc.vector.tensor_tensor(c1[:], e1[:], ps[2][:], OP.add)
                        b1 = iv("b1")
                        nc.vector.tensor_tensor(b1[:], e3[:], ps[4][:], OP.subtract)
                        d1 = iv("d1")
                        nc.vector.tensor_tensor(d1[:], e3[:], ps[4][:], OP.add)
                        t_ = iv("t_")
                        nc.vector.tensor_tensor(t_[:], ps[0][:], c1[:], OP.add)
                        tt = iv("tt")
                        nc.vector.tensor_tensor(tt[:], t_[:], d1[:], OP.add)
                        y1r = iv("y1r")
                        nc.vector.scalar_tensor_tensor(
                            y1r[:], b1[:], 2.0, a1[:], OP.mult, OP.add
                        )
                        y2r = iv("y2r")
                        nc.vector.scalar_tensor_tensor(
                            y2r[:], d1[:], 4.0, c1[:], OP.mult, OP.add
                        )
                        y3r = iv("y3r")
                        nc.vector.scalar_tensor_tensor(
                            y3r[:], b1[:], 8.0, a1[:], OP.mult, OP.add
                        )
                        y3 = iv("y3")
                        nc.vector.tensor_tensor(y3[:], y3r[:], ps[5][:], OP.add)

                        # final demod scale + pack [ty, dy, x] (ACT)
                        y = ypool.tile([P, TROWS, MO, W], F32, name="y", tag="y")
                        for dy, src_t in enumerate((tt, y1r, y2r, y3)):
                            nc.scalar.activation(
                                y[:, :, dy, :],
                                src_t[:].rearrange("p (t x) -> p t x", x=W),
                                AF.Copy,
                                scale=dsb[:, ot : ot + 1],
                            )
                        nc.sync.dma_start(
                            out[o0 : o0 + P, tch * MO * TROWS : (tch + 1) * MO * TROWS, :],
                            y[:].rearrange("p t d x -> p (t d) x"),
                        )
    nc.compile()
    return nc


_NC_CACHE = None


def _get_nc():
    global _NC_CACHE
    if _NC_CACHE is None:
        _NC_CACHE = build_nc()
    return _NC_CACHE


def make_in_maps(img, s, weight):
    """Host-side input prep: shard over batch, static weight transforms."""
    img = np.asarray(img, dtype=np.float32)
    s = np.ascontiguousarray(np.asarray(s, dtype=np.float32))
    weight = np.asarray(weight, dtype=np.float32)
    bf = ml_dtypes.bfloat16
    imgp = np.zeros((img.shape[0], NI, P, HP, WP), dtype=bf)
    imgp[:, :, :, 1 : H + 1, 1 : W + 1] = img.reshape(-1, NI, P, H, W)
    # F(4,3) G (points 0,1,-1,2,-2,inf)
    G = np.array(
        [
            [1 / 4, 0, 0],
            [-1 / 6, -1 / 6, -1 / 6],
            [-1 / 6, 1 / 6, -1 / 6],
            [1 / 24, 1 / 12, 1 / 6],
            [1 / 24, -1 / 12, 1 / 6],
            [0, 0, 1],
        ],
        np.float64,
    )
    utv = np.einsum("ag,oigx->axio", G, weight.astype(np.float64))
    utv = np.ascontiguousarray(utv.reshape(NK, CIN, COUT)).astype(bf)
    atv = np.ascontiguousarray(
        (weight.astype(np.float64) ** 2).sum(axis=(2, 3)).T
    ).astype(bf)
    return [
        {"img": imgp[b], "s": s[b], "ut": utv, "at": atv} for b in range(N_CORES)
    ]


def kernel(img, s, weight):
    nc = _get_nc()
    in_maps = make_in_maps(img, s, weight)
    res = run_bass_kernel_spmd(nc, in_maps, list(range(N_CORES)))
    return np.stack([res.results[b]["out"] for b in range(N_CORES)], axis=0)
